# revision 10
# baseline (speedup 1.0000x reference)
"""Trainium2 Bass kernel for nn_CrossAttentionFuse.

Reference computation (per batch b):
    q = Q_tokens[b] @ Wq ; k = K_tokens[b] @ Wk ; v = V_tokens[b] @ Wv   (all [T, 1024])
    per head h (16 heads x 64): kw = k_h * weight_K[b][:, None]
    S = q_h @ kw_h.T / sqrt(64) ; P = softmax(S) ; o_h = P @ v_h
    y[b] = concat_h(o_h) @ Wo + bo

Sharding (8 cores): core c handles batch b = c//2, head-group g = c%2 (8 heads,
512 feats).  Each core computes a partial y (its 8 heads' contribution to the
output projection); host sums the two partials per batch and adds bo.

Per-core layout strategy (all activations kept feature-major, "transposed"):
  - inputs are host-transposed X^T [1024, T]
  - qT, kTw [feat, tok] tiles ([128, T] x4; feat tile f = heads 2f, 2f+1)
  - kTw = k^T * weight_K (folded during PSUM eviction; removes softmax scale)
  - 1/sqrt(64) folded into Wq on host
  - v token-major [tok, feat] (tiles [128, 512] x16)
  - scores computed transposed: S^T[k, q] = kTw_chunk.T-stationary @ qT
    (2 heads row-packed per matmul pair), softmax needs no max subtraction
    (|scores| < ~2 for this problem; exp cannot overflow)
  - P~ = exp(S^T) with ACT; per-key weights already in kTw
  - PV: out^T[d, q] += v_chunk-stationary @ P~_chunk, 2 heads col-packed
  - softmax denominators = ones-vector matmul over accumulated P~ partials,
    reciprocal on DVE, partition-broadcast via SWDGE DMA, applied during the
    PSUM eviction of out^T
  - final: y[tok, out] += attT_chunk-stationary @ Wo, evict + DMA
"""

import time
from contextlib import ExitStack
from dataclasses import dataclass

import numpy as np

import concourse.bass as bass
import concourse.tile as tile
from concourse import bacc, mybir
from concourse.bass_utils import run_bass_kernel_spmd

F32 = mybir.dt.float32
F32R = mybir.dt.float32r
BF16 = mybir.dt.bfloat16
EXP = mybir.ActivationFunctionType.Exp

N_CORES = 8
NUM_HEADS = 16
HD = 64


@dataclass(frozen=True)
class Cfg:
    D: int = 1024    # input embedding dim
    T: int = 2048    # tokens (Nq == Nk)
    F: int = 512     # projected feats per core (8 heads * 64)
    OUT: int = 1024  # Wo output dim
    QTW: int = 512   # q-tile width for attention

    @property
    def IC(self):    # input-dim chunks of 128
        return self.D // 128

    @property
    def KC(self):    # key chunks of 128
        return self.T // 128

    @property
    def NF(self):    # feat tiles of 128 (head pairs)
        return self.F // 128

    @property
    def NQT(self):   # q tiles
        return self.T // self.QTW

    @property
    def PW(self):    # projection pass width (token cols per psum pass)
        return min(self.T, 1024)

    @property
    def NPASS(self):
        return self.T // self.PW


FULL = Cfg()


def emit(ctx: ExitStack, tc, io: dict, cfg: Cfg):
    nc = tc.nc
    D, T, F, OUT, QTW = cfg.D, cfg.T, cfg.F, cfg.OUT, cfg.QTW
    IC, KC, NF, NQT = cfg.IC, cfg.KC, cfg.NF, cfg.NQT
    PW, NPASS = cfg.PW, cfg.NPASS
    MMW = min(PW, 512)       # moving width per matmul in projections
    NMM = PW // MMW
    NO = max(OUT // 512, 1)  # out-dim chunks for final projection
    OCW = OUT // NO
    NQMM = max(QTW // 512, 1)  # (QTW <= 512 assumed)
    assert QTW <= 512 and OCW <= 512

    # ---------------- persistent tiles ----------------
    qkv_pool = ctx.enter_context(tc.tile_pool(name="qkv", bufs=1))
    const_pool = ctx.enter_context(tc.tile_pool(name="const", bufs=1))

    ones = const_pool.tile([128, 1], BF16, tag="ones")
    nc.vector.memset(ones, 1.0)

    qT = [qkv_pool.tile([128, T], F32R, tag=f"qT{f}", name=f"qT{f}") for f in range(NF)]
    kTw = [qkv_pool.tile([128, T], F32R, tag=f"kTw{f}", name=f"kTw{f}") for f in range(NF)]
    vtok = [qkv_pool.tile([128, F], BF16, tag=f"v{t}", name=f"v{t}") for t in range(KC)]

    # ---------------- stage A: projections ----------------
    with ExitStack() as sa:
        w_pool = sa.enter_context(tc.tile_pool(name="wproj", bufs=2))
        x_pool = sa.enter_context(tc.tile_pool(name="xchunk", bufs=3))
        pj_psum = sa.enter_context(tc.tile_pool(name="pjpsum", bufs=4, space="PSUM"))
        wkb_pool = sa.enter_context(tc.tile_pool(name="wkbp", bufs=1))

        # weight_K broadcast to all 128 partitions: [128, T]
        wkb = wkb_pool.tile([128, T], F32, tag="wkb")
        wk_ap = io["wkey"]
        wk_src = bass.AP(tensor=wk_ap.tensor, offset=wk_ap.offset,
                         ap=[[0, 128]] + list(wk_ap.ap))
        nc.gpsimd.dma_start(out=wkb[:], in_=wk_src)

        def load_w(name):
            wt = w_pool.tile([128, IC, F], F32R, tag="wproj")
            nc.sync.dma_start(
                wt[:], io[name].rearrange("(i p) f -> p i f", p=128).bitcast(F32R))
            return wt

        def proj_feat_major(xname, wname, kind):
            # q/k: out feature-major [feat, tok]; lhsT = W chunk, rhs = X^T chunk
            wt = load_w(wname)
            for pa in range(NPASS):
                ps = [pj_psum.tile([128, PW], F32, tag="pj", name=f"pj{_f}") for _f in range(NF)]
                for i in range(IC):
                    xt = x_pool.tile([128, PW], F32R, tag="xchunk")
                    nc.sync.dma_start(
                        xt[:],
                        io[xname][i * 128:(i + 1) * 128,
                                  pa * PW:(pa + 1) * PW].bitcast(F32R))
                    for f in range(NF):
                        for n in range(NMM):
                            nc.tensor.matmul(
                                ps[f][:, n * MMW:(n + 1) * MMW],
                                wt[:, i, f * 128:(f + 1) * 128],
                                xt[:, n * MMW:(n + 1) * MMW],
                                start=(i == 0), stop=(i == IC - 1))
                for f in range(NF):
                    dst_sl = slice(pa * PW, (pa + 1) * PW)
                    if kind == "q":
                        nc.scalar.copy(qT[f][:, dst_sl], ps[f][:])
                    else:  # k: fold per-key weight during eviction
                        nc.vector.tensor_mul(kTw[f][:, dst_sl], ps[f][:],
                                             wkb[:, dst_sl])

        def proj_v():
            # v: token-major [tok, feat]; lhsT = X^T chunk slice, rhs = Wv chunk.
            # X loaded in wide slabs (DMA efficiency); 4 token-subtiles per slab
            # accumulate in 4 psum tiles (reusing the "pj" slots).
            wt = load_w("wv")
            PWV = min(512, T)
            for pv in range(T // PWV):
                nts = PWV // 128
                ps = [pj_psum.tile([128, PW], F32, tag="pj", name=f"pjv{_t}")
                      for _t in range(nts)]
                for i in range(IC):
                    xt = x_pool.tile([128, PWV], F32R, tag="xv")
                    nc.sync.dma_start(
                        xt[:],
                        io["xv"][i * 128:(i + 1) * 128,
                                 pv * PWV:(pv + 1) * PWV].bitcast(F32R))
                    for ts_ in range(nts):
                        nc.tensor.matmul(
                            ps[ts_][:, 0:F],
                            xt[:, ts_ * 128:(ts_ + 1) * 128],
                            wt[:, i, 0:F],
                            start=(i == 0), stop=(i == IC - 1))
                for ts_ in range(nts):
                    nc.vector.tensor_copy(vtok[pv * nts + ts_][:],
                                          ps[ts_][:, 0:F])

        proj_feat_major("xq", "wq", "q")
        proj_feat_major("xk", "wk", "k")
        proj_v()

    # ---------------- stages B + C ----------------
    with ExitStack() as sb:
        att_pool = sb.enter_context(tc.tile_pool(name="attp", bufs=1))
        wo_pool = sb.enter_context(tc.tile_pool(name="wop", bufs=1))
        pexp_pool = sb.enter_context(tc.tile_pool(name="pexp", bufs=4))
        part_pool = sb.enter_context(tc.tile_pool(name="partials", bufs=4))
        inv_pool = sb.enter_context(tc.tile_pool(name="invp", bufs=2))
        scr_pool = sb.enter_context(tc.tile_pool(name="scrp", bufs=2, space="DRAM"))

        sbp = ExitStack()
        s_psum = sbp.enter_context(tc.tile_pool(name="spsum", bufs=2, space="PSUM"))
        o_psum = sbp.enter_context(tc.tile_pool(name="opsum", bufs=2, space="PSUM"))
        sm_psum = sbp.enter_context(tc.tile_pool(name="smpsum", bufs=2, space="PSUM"))

        attT = [att_pool.tile([128, T], F32R, tag=f"attT{f}", name=f"attT{f}") for f in range(NF)]

        wo_sb = wo_pool.tile([128, F // 128, OUT], F32R, tag="wo")
        nc.sync.dma_start(
            wo_sb[:], io["wo"].rearrange("(i p) o -> p i o", p=128).bitcast(F32R))

        for hp in range(NF):          # head pair = feat tile
            for qt in range(NQT):
                qsl = slice(qt * QTW, (qt + 1) * QTW)
                o_ps = o_psum.tile([128, QTW], F32, tag="o")
                pab_t = part_pool.tile([128, 2 * QTW], BF16, tag="pab")
                for c in range(KC):
                    csl = slice(c * 128, (c + 1) * 128)
                    s_ps = s_psum.tile([128, 2 * QTW], F32, tag="s")
                    # S^T chunk, head A (rows 0:64) and B (64:128) row-packed
                    nc.tensor.matmul(s_ps[:, 0:QTW],
                                     kTw[hp][0:64, csl], qT[hp][0:64, qsl],
                                     start=True, stop=True)
                    nc.tensor.matmul(s_ps[:, QTW:2 * QTW],
                                     kTw[hp][64:128, csl], qT[hp][64:128, qsl],
                                     start=True, stop=True)
                    # P~ = exp(S^T), both heads in one ACT op
                    pe_t = pexp_pool.tile([128, 2 * QTW], BF16, tag="pe")
                    nc.scalar.activation(pe_t[:], s_ps[:], EXP)
                    # running partial sums (for softmax denominators)
                    if c == 0:
                        nc.vector.tensor_copy(pab_t[:], pe_t[:])
                    else:
                        nc.vector.tensor_add(pab_t[:], pab_t[:], pe_t[:])
                    # out^T accumulation, 2 heads col-packed into one bank
                    nc.tensor.matmul(o_ps[0:64, :],
                                     vtok[c][:, hp * 128:hp * 128 + 64],
                                     pe_t[:, 0:QTW],
                                     start=(c == 0), stop=(c == KC - 1),
                                     skip_group_check=True)
                    nc.tensor.matmul(o_ps[64:128, :],
                                     vtok[c][:, hp * 128 + 64:hp * 128 + 128],
                                     pe_t[:, QTW:2 * QTW],
                                     start=(c == 0), stop=(c == KC - 1),
                                     skip_group_check=True)
                # denominators: sum over keys = ones^T @ partials
                sa_ps = sm_psum.tile([1, QTW], F32, tag="sm")
                sb_ps = sm_psum.tile([1, QTW], F32, tag="sm")
                nc.tensor.matmul(sa_ps[:], ones[:], pab_t[:, 0:QTW], start=True, stop=True)
                nc.tensor.matmul(sb_ps[:], ones[:], pab_t[:, QTW:2 * QTW], start=True, stop=True)
                inv1 = inv_pool.tile([1, 2 * QTW], F32, tag="inv1")
                nc.vector.reciprocal(inv1[:, 0:QTW], sa_ps[:])
                nc.vector.reciprocal(inv1[:, QTW:2 * QTW], sb_ps[:])
                # partition-broadcast 1/sums to the 64 rows of each head,
                # via a DRAM scratch roundtrip (SBUF APs cannot have
                # zero-step partition dims; DRAM APs can)
                scr = scr_pool.tile([1, 2 * QTW], F32, tag="scr")
                nc.sync.dma_start(scr[:], inv1[:])
                invb = inv_pool.tile([128, QTW], F32, tag="invb")
                scr_a = scr[:, 0:QTW]
                scr_b = scr[:, QTW:2 * QTW]
                nc.gpsimd.dma_start(
                    out=invb[0:64, :],
                    in_=bass.AP(tensor=scr_a.tensor, offset=scr_a.offset,
                                ap=[[0, 64], [1, QTW]]))
                nc.gpsimd.dma_start(
                    out=invb[64:128, :],
                    in_=bass.AP(tensor=scr_b.tensor, offset=scr_b.offset,
                                ap=[[0, 64], [1, QTW]]))
                # normalize + evict out^T into attT
                nc.vector.tensor_mul(attT[hp][:, qsl], o_ps[:], invb[:])

        sbp.close()  # release stage-B PSUM banks for stage C

        # ---------------- stage C: output projection ----------------
        with ExitStack() as sc:
            y_pool = sc.enter_context(tc.tile_pool(name="yev", bufs=2))
            y_psum = sc.enter_context(tc.tile_pool(name="ypsum", bufs=2,
                                                   space="PSUM"))
            for t in range(KC):
                tsl = slice(t * 128, (t + 1) * 128)
                y_ps = y_psum.tile([128, OUT], F32, tag="y")
                for f in range(NF):
                    for n in range(NO):
                        nc.tensor.matmul(
                            y_ps[:, n * OCW:(n + 1) * OCW],
                            attT[f][:, tsl],
                            wo_sb[:, f, n * OCW:(n + 1) * OCW],
                            start=(f == 0), stop=(f == NF - 1))
                y_sb = y_pool.tile([128, OUT], F32, tag="ysb")
                nc.scalar.copy(y_sb[:], y_ps[:])
                nc.sync.dma_start(io["y"][tsl, :], y_sb[:])


def build_program(cfg: Cfg = FULL, n_iters: int = 1):
    nc = bacc.Bacc("TRN2", target_bir_lowering=False, debug=False,
                   num_devices=N_CORES)
    io = {
        "xq": nc.dram_tensor("xq", [cfg.D, cfg.T], F32, kind="ExternalInput").ap(),
        "xk": nc.dram_tensor("xk", [cfg.D, cfg.T], F32, kind="ExternalInput").ap(),
        "xv": nc.dram_tensor("xv", [cfg.D, cfg.T], F32, kind="ExternalInput").ap(),
        "wq": nc.dram_tensor("wq", [cfg.D, cfg.F], F32, kind="ExternalInput").ap(),
        "wk": nc.dram_tensor("wk", [cfg.D, cfg.F], F32, kind="ExternalInput").ap(),
        "wv": nc.dram_tensor("wv", [cfg.D, cfg.F], F32, kind="ExternalInput").ap(),
        "wo": nc.dram_tensor("wo", [cfg.F, cfg.OUT], F32, kind="ExternalInput").ap(),
        "wkey": nc.dram_tensor("wkey", [cfg.T], F32, kind="ExternalInput").ap(),
        "y": nc.dram_tensor("y", [cfg.T, cfg.OUT], F32, kind="ExternalOutput").ap(),
    }
    with tile.TileContext(nc) as tc:
        with ExitStack() as ctx:
            if n_iters == 1:
                emit(ctx, tc, io, cfg)
            else:
                with tc.For_i(0, n_iters, 1):
                    emit(ctx, tc, io, cfg)
    nc.compile()
    return nc


def shard_inputs(Q_tokens, K_tokens, V_tokens, weight_K, Wq, Wk, Wv, Wo, bo):
    """Build the 8 per-core input maps (full -> sharded, host side)."""
    scale = 1.0 / np.sqrt(np.float32(HD))
    Wq_s = (np.asarray(Wq, np.float32) * scale)
    in_maps = []
    qt = np.ascontiguousarray(np.asarray(Q_tokens, np.float32).transpose(0, 2, 1))
    kt = np.ascontiguousarray(np.asarray(K_tokens, np.float32).transpose(0, 2, 1))
    vt = np.ascontiguousarray(np.asarray(V_tokens, np.float32).transpose(0, 2, 1))
    for c in range(N_CORES):
        b, g = divmod(c, 2)
        fs = slice(g * 512, (g + 1) * 512)
        in_maps.append({
            "xq": qt[b],
            "xk": kt[b],
            "xv": vt[b],
            "wq": np.ascontiguousarray(Wq_s[:, fs]),
            "wk": np.ascontiguousarray(np.asarray(Wk, np.float32)[:, fs]),
            "wv": np.ascontiguousarray(np.asarray(Wv, np.float32)[:, fs]),
            "wo": np.ascontiguousarray(np.asarray(Wo, np.float32)[fs, :]),
            "wkey": np.ascontiguousarray(np.asarray(weight_K, np.float32)[b]),
        })
    return in_maps


_PROGRAM_CACHE: dict = {}


def _get_program(n_iters: int = 1):
    key = n_iters
    if key not in _PROGRAM_CACHE:
        _PROGRAM_CACHE[key] = build_program(FULL, n_iters)
    return _PROGRAM_CACHE[key]


def run_sharded(in_maps, n_iters: int = 1):
    nc = _get_program(n_iters)
    t0 = time.time()
    res = run_bass_kernel_spmd(nc, in_maps, core_ids=list(range(N_CORES)))
    wall = time.time() - t0
    return res, wall


def kernel(Q_tokens, K_tokens, V_tokens, weight_K, Wq, Wk, Wv, Wo, bo):
    in_maps = shard_inputs(Q_tokens, K_tokens, V_tokens, weight_K,
                           Wq, Wk, Wv, Wo, bo)
    res, _ = run_sharded(in_maps)
    B = np.asarray(Q_tokens).shape[0]
    T, OUT = FULL.T, FULL.OUT
    out = np.empty((B, T, OUT), np.float32)
    bo = np.asarray(bo, np.float32)
    for b in range(B):
        out[b] = res.results[2 * b]["y"] + res.results[2 * b + 1]["y"] + bo
    return out


# revision 15
# speedup vs baseline: 1.1638x; 1.1638x over previous
"""Trainium2 Bass kernel for nn_CrossAttentionFuse.

Reference computation (per batch b):
    q = Q_tokens[b] @ Wq ; k = K_tokens[b] @ Wk ; v = V_tokens[b] @ Wv   (all [T, 1024])
    per head h (16 heads x 64): kw = k_h * weight_K[b][:, None]
    S = q_h @ kw_h.T / sqrt(64) ; P = softmax(S) ; o_h = P @ v_h
    y[b] = concat_h(o_h) @ Wo + bo

Sharding (8 cores): core c handles batch b = c//2, head-group g = c%2 (8 heads,
512 feats).  Each core computes a partial y (its 8 heads' contribution to the
output projection); host sums the two partials per batch and adds bo.

Per-core layout strategy (all activations kept feature-major, "transposed"):
  - inputs are host-transposed X^T [1024, T]
  - qT, kTw [feat, tok] tiles ([128, T] x4; feat tile f = heads 2f, 2f+1)
  - kTw = k^T * weight_K (folded during PSUM eviction; removes softmax scale)
  - 1/sqrt(64) folded into Wq on host
  - v token-major [tok, feat] (tiles [128, 512] x16)
  - scores computed transposed: S^T[k, q] = kTw_chunk.T-stationary @ qT
    (2 heads row-packed per matmul pair), softmax needs no max subtraction
    (|scores| < ~2 for this problem; exp cannot overflow)
  - P~ = exp(S^T) with ACT; per-key weights already in kTw
  - PV: out^T[d, q] += v_chunk-stationary @ P~_chunk, 2 heads col-packed
  - softmax denominators = ones-vector matmul over accumulated P~ partials,
    reciprocal on DVE, partition-broadcast via SWDGE DMA, applied during the
    PSUM eviction of out^T
  - final: y[tok, out] += attT_chunk-stationary @ Wo, evict + DMA
"""

import time
from contextlib import ExitStack
from dataclasses import dataclass

import numpy as np

import concourse.bass as bass
import concourse.tile as tile
from concourse import bacc, mybir
from concourse.bass_utils import run_bass_kernel_spmd

F32 = mybir.dt.float32
F32R = mybir.dt.float32r
BF16 = mybir.dt.bfloat16
EXP = mybir.ActivationFunctionType.Exp

N_CORES = 8
NUM_HEADS = 16
HD = 64


@dataclass(frozen=True)
class Cfg:
    D: int = 1024    # input embedding dim
    T: int = 2048    # tokens (Nq == Nk)
    F: int = 512     # projected feats per core (8 heads * 64)
    OUT: int = 1024  # Wo output dim
    QTW: int = 512   # q-tile width for attention

    @property
    def IC(self):    # input-dim chunks of 128
        return self.D // 128

    @property
    def KC(self):    # key chunks of 128
        return self.T // 128

    @property
    def NF(self):    # feat tiles of 128 (head pairs)
        return self.F // 128

    @property
    def NQT(self):   # q tiles
        return self.T // self.QTW

    @property
    def PW(self):    # projection pass width (token cols per psum pass)
        return min(self.T, 1024)

    @property
    def NPASS(self):
        return self.T // self.PW


FULL = Cfg()


def emit(ctx: ExitStack, tc, io: dict, cfg: Cfg, blevel: int = 4, do_c: bool = True):
    nc = tc.nc
    D, T, F, OUT, QTW = cfg.D, cfg.T, cfg.F, cfg.OUT, cfg.QTW
    IC, KC, NF, NQT = cfg.IC, cfg.KC, cfg.NF, cfg.NQT
    PW, NPASS = cfg.PW, cfg.NPASS
    MMW = min(PW, 512)       # moving width per matmul in projections
    NMM = PW // MMW
    NO = max(OUT // 512, 1)  # out-dim chunks for final projection
    OCW = OUT // NO
    NQMM = max(QTW // 512, 1)  # (QTW <= 512 assumed)
    assert QTW <= 512 and OCW <= 512

    # ---------------- persistent tiles ----------------
    qkv_pool = ctx.enter_context(tc.tile_pool(name="qkv", bufs=1))
    const_pool = ctx.enter_context(tc.tile_pool(name="const", bufs=1))

    ones = const_pool.tile([128, 1], BF16, tag="ones")
    nc.vector.memset(ones, 1.0)

    qT = [qkv_pool.tile([128, T], BF16, tag=f"qT{f}", name=f"qT{f}") for f in range(NF)]
    kTw = [qkv_pool.tile([128, T], BF16, tag=f"kTw{f}", name=f"kTw{f}") for f in range(NF)]
    vtok = [qkv_pool.tile([128, F], BF16, tag=f"v{t}", name=f"v{t}") for t in range(KC)]

    # ---------------- stage A: projections ----------------
    with ExitStack() as sa:
        w_pool = sa.enter_context(tc.tile_pool(name="wproj", bufs=2))
        x_pool = sa.enter_context(tc.tile_pool(name="xchunk", bufs=3))
        pj_psum = sa.enter_context(tc.tile_pool(name="pjpsum", bufs=4, space="PSUM"))
        wkb_pool = sa.enter_context(tc.tile_pool(name="wkbp", bufs=1))

        # weight_K broadcast to all 128 partitions: [128, T]
        wkb = wkb_pool.tile([128, T], F32, tag="wkb")
        wk_ap = io["wkey"]
        wk_src = bass.AP(tensor=wk_ap.tensor, offset=wk_ap.offset,
                         ap=[[0, 128]] + list(wk_ap.ap))
        nc.gpsimd.dma_start(out=wkb[:], in_=wk_src)

        def load_w(name):
            wt = w_pool.tile([128, IC, F], BF16, tag="wproj")
            nc.sync.dma_start(
                wt[:], io[name].rearrange("(i p) f -> p i f", p=128))
            return wt

        def proj_feat_major(xname, wname, kind):
            # q/k: out feature-major [feat, tok]; lhsT = W chunk, rhs = X^T chunk
            wt = load_w(wname)
            for pa in range(NPASS):
                ps = [pj_psum.tile([128, PW], F32, tag="pj", name=f"pj{_f}") for _f in range(NF)]
                for i in range(IC):
                    xt = x_pool.tile([128, PW], BF16, tag="xchunk")
                    nc.sync.dma_start(
                        xt[:],
                        io[xname][i * 128:(i + 1) * 128,
                                  pa * PW:(pa + 1) * PW])
                    for f in range(NF):
                        for n in range(NMM):
                            nc.tensor.matmul(
                                ps[f][:, n * MMW:(n + 1) * MMW],
                                wt[:, i, f * 128:(f + 1) * 128],
                                xt[:, n * MMW:(n + 1) * MMW],
                                start=(i == 0), stop=(i == IC - 1))
                for f in range(NF):
                    dst_sl = slice(pa * PW, (pa + 1) * PW)
                    if kind == "q":
                        nc.scalar.copy(qT[f][:, dst_sl], ps[f][:])
                    else:  # k: fold per-key weight during eviction
                        nc.vector.tensor_mul(kTw[f][:, dst_sl], ps[f][:],
                                             wkb[:, dst_sl])

        def proj_v():
            # v: token-major [tok, feat]; lhsT = X^T chunk slice, rhs = Wv chunk.
            # X loaded in wide slabs (DMA efficiency); 4 token-subtiles per slab
            # accumulate in 4 psum tiles (reusing the "pj" slots).
            wt = load_w("wv")
            PWV = min(512, T)
            for pv in range(T // PWV):
                nts = PWV // 128
                ps = [pj_psum.tile([128, PW], F32, tag="pj", name=f"pjv{_t}")
                      for _t in range(nts)]
                for i in range(IC):
                    xt = x_pool.tile([128, PWV], BF16, tag="xv")
                    nc.sync.dma_start(
                        xt[:],
                        io["xv"][i * 128:(i + 1) * 128,
                                 pv * PWV:(pv + 1) * PWV])
                    for ts_ in range(nts):
                        nc.tensor.matmul(
                            ps[ts_][:, 0:F],
                            xt[:, ts_ * 128:(ts_ + 1) * 128],
                            wt[:, i, 0:F],
                            start=(i == 0), stop=(i == IC - 1))
                for ts_ in range(nts):
                    nc.vector.tensor_copy(vtok[pv * nts + ts_][:],
                                          ps[ts_][:, 0:F])

        proj_feat_major("xq", "wq", "q")
        proj_feat_major("xk", "wk", "k")
        proj_v()

    if blevel == 0:
        return

    # ---------------- stages B + C ----------------
    with ExitStack() as sb:
        att_pool = sb.enter_context(tc.tile_pool(name="attp", bufs=1))
        wo_pool = sb.enter_context(tc.tile_pool(name="wop", bufs=1))
        pexp_pool = sb.enter_context(tc.tile_pool(name="pexp", bufs=4))
        part_pool = sb.enter_context(tc.tile_pool(name="partials", bufs=4))
        inv_pool = sb.enter_context(tc.tile_pool(name="invp", bufs=2))
        scr_pool = sb.enter_context(tc.tile_pool(name="scrp", bufs=2, space="DRAM"))

        sbp = ExitStack()
        s_psum = sbp.enter_context(tc.tile_pool(name="spsum", bufs=2, space="PSUM"))
        o_psum = sbp.enter_context(tc.tile_pool(name="opsum", bufs=2, space="PSUM"))
        sm_psum = sbp.enter_context(tc.tile_pool(name="smpsum", bufs=2, space="PSUM"))

        attT = [att_pool.tile([128, T], BF16, tag=f"attT{f}", name=f"attT{f}") for f in range(NF)]

        wo_sb = wo_pool.tile([128, F // 128, OUT], BF16, tag="wo")
        nc.sync.dma_start(
            wo_sb[:], io["wo"].rearrange("(i p) o -> p i o", p=128))

        def emit_pv(hp, c, pe_t, o_ps):
            # out^T accumulation, 2 heads col-packed into one bank
            nc.tensor.matmul(o_ps[0:64, :],
                             vtok[c][:, hp * 128:hp * 128 + 64],
                             pe_t[:, 0:QTW],
                             start=(c == 0), stop=(c == KC - 1),
                             skip_group_check=True)
            nc.tensor.matmul(o_ps[64:128, :],
                             vtok[c][:, hp * 128 + 64:hp * 128 + 128],
                             pe_t[:, QTW:2 * QTW],
                             start=(c == 0), stop=(c == KC - 1),
                             skip_group_check=True)

        def norm_tail(hp, qt, o_ps, pab_t):
            # softmax denominators + normalization for one finished (hp, qt)
            qsl = slice(qt * QTW, (qt + 1) * QTW)
            sa_ps = sm_psum.tile([1, QTW], F32, tag="sm")
            sb_ps = sm_psum.tile([1, QTW], F32, tag="sm")
            nc.tensor.matmul(sa_ps[:], ones[:], pab_t[:, 0:QTW],
                             start=True, stop=True)
            nc.tensor.matmul(sb_ps[:], ones[:], pab_t[:, QTW:2 * QTW],
                             start=True, stop=True)
            inv1 = inv_pool.tile([1, 2 * QTW], F32, tag="inv1")
            nc.vector.reciprocal(inv1[:, 0:QTW], sa_ps[:])
            nc.vector.reciprocal(inv1[:, QTW:2 * QTW], sb_ps[:])
            # partition-broadcast 1/sums to the 64 rows of each head, via a
            # DRAM scratch roundtrip (SBUF APs cannot have zero-step
            # partition dims; DRAM APs can)
            scr = scr_pool.tile([1, 2 * QTW], F32, tag="scr")
            nc.sync.dma_start(scr[:], inv1[:])
            invb = inv_pool.tile([128, QTW], F32, tag="invb")
            scr_a = scr[:, 0:QTW]
            scr_b = scr[:, QTW:2 * QTW]
            nc.gpsimd.dma_start(
                out=invb[0:64, :],
                in_=bass.AP(tensor=scr_a.tensor, offset=scr_a.offset,
                            ap=[[0, 64], [1, QTW]]))
            nc.gpsimd.dma_start(
                out=invb[64:128, :],
                in_=bass.AP(tensor=scr_b.tensor, offset=scr_b.offset,
                            ap=[[0, 64], [1, QTW]]))
            # normalize + evict out^T into attT
            nc.vector.tensor_mul(attT[hp][:, qsl], o_ps[:], invb[:])

        pending = None  # deferred norm_tail args from the previous (hp, qt)
        for hp in range(NF):          # head pair = feat tile
            for qt in range(NQT):
                qsl = slice(qt * QTW, (qt + 1) * QTW)
                o_ps = o_psum.tile([128, QTW], F32, tag="o")
                pab_t = part_pool.tile([128, 2 * QTW], BF16, tag="pab")
                prev_pv = None
                for c in range(KC):
                    csl = slice(c * 128, (c + 1) * 128)
                    s_ps = s_psum.tile([128, 2 * QTW], F32, tag="s")
                    # S^T chunk, head A (rows 0:64) and B (64:128) row-packed
                    nc.tensor.matmul(s_ps[:, 0:QTW],
                                     kTw[hp][0:64, csl], qT[hp][0:64, qsl],
                                     start=True, stop=True)
                    nc.tensor.matmul(s_ps[:, QTW:2 * QTW],
                                     kTw[hp][64:128, csl], qT[hp][64:128, qsl],
                                     start=True, stop=True)
                    if blevel >= 2:
                        # P~ = exp(S^T), both heads in one ACT op
                        pe_t = pexp_pool.tile([128, 2 * QTW], BF16, tag="pe")
                        nc.scalar.activation(pe_t[:], s_ps[:], EXP)
                        # running partial sums (softmax denominators)
                        if blevel >= 4 and c == 0:
                            nc.vector.tensor_copy(pab_t[:], pe_t[:])
                        elif blevel >= 4:
                            nc.vector.tensor_add(pab_t[:], pab_t[:], pe_t[:])
                        if blevel >= 3:
                            # PV deferred one chunk: QK(c+1) goes ahead of
                            # PV(c) in the PE stream, so the PE never stalls
                            # on exp(c) with independent work still queued
                            if prev_pv is not None:
                                pc, ppe = prev_pv
                                emit_pv(hp, pc, ppe, o_ps)
                            prev_pv = (c, pe_t)
                    if c == min(2, KC - 1) and pending is not None:
                        norm_tail(*pending)
                        pending = None
                if prev_pv is not None:
                    pc, ppe = prev_pv
                    emit_pv(hp, pc, ppe, o_ps)
                if blevel >= 4:
                    pending = (hp, qt, o_ps, pab_t)
        if pending is not None:
            norm_tail(*pending)

        sbp.close()  # release stage-B PSUM banks for stage C

        if not do_c or blevel < 4:
            return
        # ---------------- stage C: output projection ----------------
        with ExitStack() as sc:
            y_pool = sc.enter_context(tc.tile_pool(name="yev", bufs=2))
            y_psum = sc.enter_context(tc.tile_pool(name="ypsum", bufs=2,
                                                   space="PSUM"))
            for t in range(KC):
                tsl = slice(t * 128, (t + 1) * 128)
                y_ps = y_psum.tile([128, OUT], F32, tag="y")
                for f in range(NF):
                    for n in range(NO):
                        nc.tensor.matmul(
                            y_ps[:, n * OCW:(n + 1) * OCW],
                            attT[f][:, tsl],
                            wo_sb[:, f, n * OCW:(n + 1) * OCW],
                            start=(f == 0), stop=(f == NF - 1))
                y_sb = y_pool.tile([128, OUT], F32, tag="ysb")
                nc.scalar.copy(y_sb[:], y_ps[:])
                nc.sync.dma_start(io["y"][tsl, :], y_sb[:])


def build_program(cfg: Cfg = FULL, n_iters: int = 1, blevel: int = 4, do_c: bool = True):
    nc = bacc.Bacc("TRN2", target_bir_lowering=False, debug=False,
                   num_devices=N_CORES)
    io = {
        "xq": nc.dram_tensor("xq", [cfg.D, cfg.T], BF16, kind="ExternalInput").ap(),
        "xk": nc.dram_tensor("xk", [cfg.D, cfg.T], BF16, kind="ExternalInput").ap(),
        "xv": nc.dram_tensor("xv", [cfg.D, cfg.T], BF16, kind="ExternalInput").ap(),
        "wq": nc.dram_tensor("wq", [cfg.D, cfg.F], BF16, kind="ExternalInput").ap(),
        "wk": nc.dram_tensor("wk", [cfg.D, cfg.F], BF16, kind="ExternalInput").ap(),
        "wv": nc.dram_tensor("wv", [cfg.D, cfg.F], BF16, kind="ExternalInput").ap(),
        "wo": nc.dram_tensor("wo", [cfg.F, cfg.OUT], BF16, kind="ExternalInput").ap(),
        "wkey": nc.dram_tensor("wkey", [cfg.T], F32, kind="ExternalInput").ap(),
        "y": nc.dram_tensor("y", [cfg.T, cfg.OUT], F32, kind="ExternalOutput").ap(),
    }
    with tile.TileContext(nc) as tc:
        with ExitStack() as ctx:
            if n_iters == 1:
                emit(ctx, tc, io, cfg, blevel, do_c)
            else:
                with tc.For_i(0, n_iters, 1):
                    emit(ctx, tc, io, cfg, blevel, do_c)
    nc.compile()
    return nc


def shard_inputs(Q_tokens, K_tokens, V_tokens, weight_K, Wq, Wk, Wv, Wo, bo):
    """Build the 8 per-core input maps (full -> sharded, host side)."""
    import ml_dtypes
    bf = ml_dtypes.bfloat16
    scale = 1.0 / np.sqrt(np.float32(HD))
    Wq_s = (np.asarray(Wq, np.float32) * scale)
    in_maps = []
    qt = np.ascontiguousarray(np.asarray(Q_tokens, np.float32).transpose(0, 2, 1)).astype(bf)
    kt = np.ascontiguousarray(np.asarray(K_tokens, np.float32).transpose(0, 2, 1)).astype(bf)
    vt = np.ascontiguousarray(np.asarray(V_tokens, np.float32).transpose(0, 2, 1)).astype(bf)
    for c in range(N_CORES):
        b, g = divmod(c, 2)
        fs = slice(g * 512, (g + 1) * 512)
        in_maps.append({
            "xq": qt[b],
            "xk": kt[b],
            "xv": vt[b],
            "wq": np.ascontiguousarray(Wq_s[:, fs]).astype(bf),
            "wk": np.ascontiguousarray(np.asarray(Wk, np.float32)[:, fs]).astype(bf),
            "wv": np.ascontiguousarray(np.asarray(Wv, np.float32)[:, fs]).astype(bf),
            "wo": np.ascontiguousarray(np.asarray(Wo, np.float32)[fs, :]).astype(bf),
            "wkey": np.ascontiguousarray(np.asarray(weight_K, np.float32)[b]),
        })
    return in_maps


_PROGRAM_CACHE: dict = {}


def _get_program(n_iters: int = 1, blevel: int = 4, do_c: bool = True):
    key = (n_iters, blevel, do_c)
    if key not in _PROGRAM_CACHE:
        _PROGRAM_CACHE[key] = build_program(FULL, n_iters, blevel, do_c)
    return _PROGRAM_CACHE[key]


def run_sharded(in_maps, n_iters: int = 1):
    nc = _get_program(n_iters)
    t0 = time.time()
    res = run_bass_kernel_spmd(nc, in_maps, core_ids=list(range(N_CORES)))
    wall = time.time() - t0
    return res, wall


def kernel(Q_tokens, K_tokens, V_tokens, weight_K, Wq, Wk, Wv, Wo, bo):
    in_maps = shard_inputs(Q_tokens, K_tokens, V_tokens, weight_K,
                           Wq, Wk, Wv, Wo, bo)
    res, _ = run_sharded(in_maps)
    B = np.asarray(Q_tokens).shape[0]
    T, OUT = FULL.T, FULL.OUT
    out = np.empty((B, T, OUT), np.float32)
    bo = np.asarray(bo, np.float32)
    for b in range(B):
        out[b] = res.results[2 * b]["y"] + res.results[2 * b + 1]["y"] + bo
    return out


# revision 19
# speedup vs baseline: 1.2105x; 1.0402x over previous
"""Trainium2 Bass kernel for nn_CrossAttentionFuse.

Reference computation (per batch b):
    q = Q_tokens[b] @ Wq ; k = K_tokens[b] @ Wk ; v = V_tokens[b] @ Wv   (all [T, 1024])
    per head h (16 heads x 64): kw = k_h * weight_K[b][:, None]
    S = q_h @ kw_h.T / sqrt(64) ; P = softmax(S) ; o_h = P @ v_h
    y[b] = concat_h(o_h) @ Wo + bo

Sharding (8 cores): core c handles batch b = c//2, head-group g = c%2 (8 heads,
512 feats).  Each core computes a partial y (its 8 heads' contribution to the
output projection); host sums the two partials per batch and adds bo.

Per-core layout strategy (all activations kept feature-major, "transposed"):
  - inputs are host-transposed X^T [1024, T]
  - qT, kTw [feat, tok] tiles ([128, T] x4; feat tile f = heads 2f, 2f+1)
  - kTw = k^T * weight_K (folded during PSUM eviction; removes softmax scale)
  - 1/sqrt(64) folded into Wq on host
  - v token-major [tok, feat] (tiles [128, 512] x16)
  - scores computed transposed: S^T[k, q] = kTw_chunk.T-stationary @ qT
    (2 heads row-packed per matmul pair), softmax needs no max subtraction
    (|scores| < ~2 for this problem; exp cannot overflow)
  - P~ = exp(S^T) with ACT; per-key weights already in kTw
  - PV: out^T[d, q] += v_chunk-stationary @ P~_chunk, 2 heads col-packed
  - softmax denominators = ones-vector matmul over accumulated P~ partials,
    reciprocal on DVE, partition-broadcast via SWDGE DMA, applied during the
    PSUM eviction of out^T
  - final: y[tok, out] += attT_chunk-stationary @ Wo, evict + DMA
"""

import time
from contextlib import ExitStack
from dataclasses import dataclass

import numpy as np

import concourse.bass as bass
import concourse.tile as tile
from concourse import bacc, mybir
from concourse.bass_utils import run_bass_kernel_spmd

F32 = mybir.dt.float32
F32R = mybir.dt.float32r
BF16 = mybir.dt.bfloat16
EXP = mybir.ActivationFunctionType.Exp

N_CORES = 8
NUM_HEADS = 16
HD = 64


@dataclass(frozen=True)
class Cfg:
    D: int = 1024    # input embedding dim
    T: int = 2048    # tokens (Nq == Nk)
    F: int = 512     # projected feats per core (8 heads * 64)
    OUT: int = 1024  # Wo output dim
    QTW: int = 512   # q-tile width for attention

    @property
    def IC(self):    # input-dim chunks of 128
        return self.D // 128

    @property
    def KC(self):    # key chunks of 128
        return self.T // 128

    @property
    def NF(self):    # feat tiles of 128 (head pairs)
        return self.F // 128

    @property
    def NQT(self):   # q tiles
        return self.T // self.QTW

    @property
    def PW(self):    # projection pass width (token cols per psum pass)
        return min(self.T, 1024)

    @property
    def NPASS(self):
        return self.T // self.PW


FULL = Cfg()


def emit(ctx: ExitStack, tc, io: dict, cfg: Cfg, blevel: int = 4, do_c: bool = True):
    nc = tc.nc
    D, T, F, OUT, QTW = cfg.D, cfg.T, cfg.F, cfg.OUT, cfg.QTW
    IC, KC, NF, NQT = cfg.IC, cfg.KC, cfg.NF, cfg.NQT
    PW, NPASS = cfg.PW, cfg.NPASS
    MMW = min(PW, 512)       # moving width per matmul in projections
    NMM = PW // MMW
    NO = max(OUT // 512, 1)  # out-dim chunks for final projection
    OCW = OUT // NO
    NQMM = max(QTW // 512, 1)  # (QTW <= 512 assumed)
    assert QTW <= 512 and OCW <= 512

    # ---------------- persistent tiles ----------------
    qkv_pool = ctx.enter_context(tc.tile_pool(name="qkv", bufs=1))
    const_pool = ctx.enter_context(tc.tile_pool(name="const", bufs=1))

    ones = const_pool.tile([128, 1], BF16, tag="ones")
    nc.vector.memset(ones, 1.0)

    qT = [qkv_pool.tile([128, T], BF16, tag=f"qT{f}", name=f"qT{f}") for f in range(NF)]
    kTw = [qkv_pool.tile([128, T], BF16, tag=f"kTw{f}", name=f"kTw{f}") for f in range(NF)]
    vtok = [qkv_pool.tile([128, F], BF16, tag=f"v{t}", name=f"v{t}") for t in range(KC)]

    # ---------------- stage A: projections ----------------
    with ExitStack() as sa:
        w_pool = sa.enter_context(tc.tile_pool(name="wproj", bufs=3))
        x_pool = sa.enter_context(tc.tile_pool(name="xchunk", bufs=5))
        pj_psum = sa.enter_context(tc.tile_pool(name="pjpsum", bufs=4, space="PSUM"))
        wkb_pool = sa.enter_context(tc.tile_pool(name="wkbp", bufs=1))

        # weight_K broadcast to all 128 partitions: [128, T]
        wkb = wkb_pool.tile([128, T], F32, tag="wkb")
        wk_ap = io["wkey"]
        wk_src = bass.AP(tensor=wk_ap.tensor, offset=wk_ap.offset,
                         ap=[[0, 128]] + list(wk_ap.ap))
        nc.gpsimd.dma_start(out=wkb[:], in_=wk_src)

        def load_w(name):
            wt = w_pool.tile([128, IC, F], BF16, tag="wproj")
            nc.sync.dma_start(
                wt[:], io[name].rearrange("(i p) f -> p i f", p=128))
            return wt

        def proj_feat_major(xname, wname, kind):
            # q/k: out feature-major [feat, tok]; lhsT = W chunk, rhs = X^T chunk
            wt = load_w(wname)
            for pa in range(NPASS):
                ps = [pj_psum.tile([128, PW], F32, tag="pj", name=f"pj{_f}") for _f in range(NF)]
                for i in range(IC):
                    xt = x_pool.tile([128, PW], BF16, tag="xchunk")
                    nc.sync.dma_start(
                        xt[:],
                        io[xname][i * 128:(i + 1) * 128,
                                  pa * PW:(pa + 1) * PW])
                    for f in range(NF):
                        for n in range(NMM):
                            nc.tensor.matmul(
                                ps[f][:, n * MMW:(n + 1) * MMW],
                                wt[:, i, f * 128:(f + 1) * 128],
                                xt[:, n * MMW:(n + 1) * MMW],
                                start=(i == 0), stop=(i == IC - 1))
                for f in range(NF):
                    dst_sl = slice(pa * PW, (pa + 1) * PW)
                    if kind == "q":
                        nc.scalar.copy(qT[f][:, dst_sl], ps[f][:])
                    else:  # k: fold per-key weight during eviction
                        nc.vector.tensor_mul(kTw[f][:, dst_sl], ps[f][:],
                                             wkb[:, dst_sl])

        def proj_v():
            # v: token-major [tok, feat]; lhsT = X^T chunk slice, rhs = Wv chunk.
            # X loaded in wide slabs (DMA efficiency); 4 token-subtiles per slab
            # accumulate in 4 psum tiles (reusing the "pj" slots).
            wt = load_w("wv")
            PWV = min(512, T)
            for pv in range(T // PWV):
                nts = PWV // 128
                ps = [pj_psum.tile([128, PW], F32, tag="pj", name=f"pjv{_t}")
                      for _t in range(nts)]
                for i in range(IC):
                    xt = x_pool.tile([128, PWV], BF16, tag="xv")
                    nc.sync.dma_start(
                        xt[:],
                        io["xv"][i * 128:(i + 1) * 128,
                                 pv * PWV:(pv + 1) * PWV])
                    for ts_ in range(nts):
                        nc.tensor.matmul(
                            ps[ts_][:, 0:F],
                            xt[:, ts_ * 128:(ts_ + 1) * 128],
                            wt[:, i, 0:F],
                            start=(i == 0), stop=(i == IC - 1))
                for ts_ in range(nts):
                    nc.vector.tensor_copy(vtok[pv * nts + ts_][:],
                                          ps[ts_][:, 0:F])

        proj_feat_major("xq", "wq", "q")
        proj_feat_major("xk", "wk", "k")
        proj_v()

    if blevel == 0:
        return

    # ---------------- stages B + C ----------------
    with ExitStack() as sb:
        att_pool = sb.enter_context(tc.tile_pool(name="attp", bufs=1))
        wo_pool = sb.enter_context(tc.tile_pool(name="wop", bufs=1))
        pexp_pool = sb.enter_context(tc.tile_pool(name="pexp", bufs=4))
        part_pool = sb.enter_context(tc.tile_pool(name="partials", bufs=4))
        inv_pool = sb.enter_context(tc.tile_pool(name="invp", bufs=2))
        scr_pool = sb.enter_context(tc.tile_pool(name="scrp", bufs=2, space="DRAM"))

        sbp = ExitStack()
        s_psum = sbp.enter_context(tc.tile_pool(name="spsum", bufs=2, space="PSUM"))
        o_psum = sbp.enter_context(tc.tile_pool(name="opsum", bufs=2, space="PSUM"))
        sm_psum = sbp.enter_context(tc.tile_pool(name="smpsum", bufs=2, space="PSUM"))

        attT = [att_pool.tile([128, T], BF16, tag=f"attT{f}", name=f"attT{f}") for f in range(NF)]

        wo_sb = wo_pool.tile([128, F // 128, OUT], BF16, tag="wo")
        nc.sync.dma_start(
            wo_sb[:], io["wo"].rearrange("(i p) o -> p i o", p=128))

        def emit_pv(hp, c, pe_t, o_ps):
            # out^T accumulation, 2 heads col-packed into one bank
            nc.tensor.matmul(o_ps[0:64, :],
                             vtok[c][:, hp * 128:hp * 128 + 64],
                             pe_t[:, 0:QTW],
                             start=(c == 0), stop=(c == KC - 1),
                             skip_group_check=True)
            nc.tensor.matmul(o_ps[64:128, :],
                             vtok[c][:, hp * 128 + 64:hp * 128 + 128],
                             pe_t[:, QTW:2 * QTW],
                             start=(c == 0), stop=(c == KC - 1),
                             skip_group_check=True)

        def norm_tail_a(hp, qt, o_ps, pab_t):
            # denominators for one finished (hp, qt): sums -> 1/sums -> start
            # the partition-broadcast (DRAM roundtrip; SBUF APs cannot have
            # zero-step partition dims, DRAM APs can)
            sm_ps = sm_psum.tile([128, QTW], F32, tag="sm")
            nc.tensor.matmul(sm_ps[0:1, :], ones[:], pab_t[:, 0:QTW],
                             start=True, stop=True, skip_group_check=True)
            nc.tensor.matmul(sm_ps[32:33, :], ones[:], pab_t[:, QTW:2 * QTW],
                             start=True, stop=True, skip_group_check=True)
            inv1 = inv_pool.tile([1, 2 * QTW], F32, tag="inv1")
            nc.vector.reciprocal(inv1[:, 0:QTW], sm_ps[0:1, :])
            nc.vector.reciprocal(inv1[:, QTW:2 * QTW], sm_ps[32:33, :])
            scr = scr_pool.tile([1, 2 * QTW], F32, tag="scr")
            nc.sync.dma_start(scr[:], inv1[:])
            invb = inv_pool.tile([128, QTW], F32, tag="invb")
            scr_a = scr[:, 0:QTW]
            scr_b = scr[:, QTW:2 * QTW]
            nc.gpsimd.dma_start(
                out=invb[0:64, :],
                in_=bass.AP(tensor=scr_a.tensor, offset=scr_a.offset,
                            ap=[[0, 64], [1, QTW]]))
            nc.gpsimd.dma_start(
                out=invb[64:128, :],
                in_=bass.AP(tensor=scr_b.tensor, offset=scr_b.offset,
                            ap=[[0, 64], [1, QTW]]))
            return invb

        def norm_tail_b(hp, qt, o_ps, invb):
            # normalize + evict out^T into attT (deferred so the DVE stream
            # doesn't stall on the broadcast-DMA latency)
            qsl = slice(qt * QTW, (qt + 1) * QTW)
            nc.vector.tensor_mul(attT[hp][:, qsl], o_ps[:], invb[:])

        TA = min(2, KC - 1)   # chunk at which the previous tail's sums start
        TB = min(10, KC - 1)  # chunk at which the previous tail's mul runs
        pending = None  # deferred norm_tail args from the previous (hp, qt)
        pending_b = None
        for hp in range(NF):          # head pair = feat tile
            for qt in range(NQT):
                qsl = slice(qt * QTW, (qt + 1) * QTW)
                o_ps = o_psum.tile([128, QTW], F32, tag="o")
                pab_t = part_pool.tile([128, 2 * QTW], BF16, tag="pab")
                prev_pv = None
                for c in range(KC):
                    csl = slice(c * 128, (c + 1) * 128)
                    s_ps = s_psum.tile([128, 2 * QTW], F32, tag="s")
                    # S^T chunk, head A (rows 0:64) and B (64:128) row-packed
                    nc.tensor.matmul(s_ps[:, 0:QTW],
                                     kTw[hp][0:64, csl], qT[hp][0:64, qsl],
                                     start=True, stop=True)
                    nc.tensor.matmul(s_ps[:, QTW:2 * QTW],
                                     kTw[hp][64:128, csl], qT[hp][64:128, qsl],
                                     start=True, stop=True)
                    if blevel >= 2:
                        # P~ = exp(S^T), both heads in one ACT op
                        pe_t = pexp_pool.tile([128, 2 * QTW], BF16, tag="pe")
                        nc.scalar.activation(pe_t[:], s_ps[:], EXP)
                        # running partial sums (softmax denominators)
                        if blevel >= 4 and c == 0:
                            nc.vector.tensor_copy(pab_t[:], pe_t[:])
                        elif blevel >= 4:
                            nc.vector.tensor_add(pab_t[:], pab_t[:], pe_t[:])
                        if blevel >= 3:
                            # PV deferred one chunk: QK(c+1) goes ahead of
                            # PV(c) in the PE stream, so the PE never stalls
                            # on exp(c) with independent work still queued
                            if prev_pv is not None:
                                pc, ppe = prev_pv
                                emit_pv(hp, pc, ppe, o_ps)
                            prev_pv = (c, pe_t)
                    if c == TA and pending is not None:
                        ihp, iqt, io_ps, ipab = pending
                        invb = norm_tail_a(ihp, iqt, io_ps, ipab)
                        pending_b = (ihp, iqt, io_ps, invb)
                        pending = None
                    if c == TB and pending_b is not None:
                        norm_tail_b(*pending_b)
                        pending_b = None
                if prev_pv is not None:
                    pc, ppe = prev_pv
                    emit_pv(hp, pc, ppe, o_ps)
                if blevel >= 4:
                    pending = (hp, qt, o_ps, pab_t)
        if pending is not None:
            ihp, iqt, io_ps, ipab = pending
            invb = norm_tail_a(ihp, iqt, io_ps, ipab)
            pending_b = (ihp, iqt, io_ps, invb)
        if pending_b is not None:
            norm_tail_b(*pending_b)

        sbp.close()  # release stage-B PSUM banks for stage C

        if not do_c or blevel < 4:
            return
        # ---------------- stage C: output projection ----------------
        with ExitStack() as sc:
            y_pool = sc.enter_context(tc.tile_pool(name="yev", bufs=2))
            y_psum = sc.enter_context(tc.tile_pool(name="ypsum", bufs=2,
                                                   space="PSUM"))
            for t in range(KC):
                tsl = slice(t * 128, (t + 1) * 128)
                y_ps = y_psum.tile([128, OUT], F32, tag="y")
                for f in range(NF):
                    for n in range(NO):
                        nc.tensor.matmul(
                            y_ps[:, n * OCW:(n + 1) * OCW],
                            attT[f][:, tsl],
                            wo_sb[:, f, n * OCW:(n + 1) * OCW],
                            start=(f == 0), stop=(f == NF - 1))
                y_sb = y_pool.tile([128, OUT], F32, tag="ysb")
                nc.scalar.copy(y_sb[:], y_ps[:])
                nc.sync.dma_start(io["y"][tsl, :], y_sb[:])


def build_program(cfg: Cfg = FULL, n_iters: int = 1, blevel: int = 4, do_c: bool = True):
    nc = bacc.Bacc("TRN2", target_bir_lowering=False, debug=False,
                   num_devices=N_CORES)
    io = {
        "xq": nc.dram_tensor("xq", [cfg.D, cfg.T], BF16, kind="ExternalInput").ap(),
        "xk": nc.dram_tensor("xk", [cfg.D, cfg.T], BF16, kind="ExternalInput").ap(),
        "xv": nc.dram_tensor("xv", [cfg.D, cfg.T], BF16, kind="ExternalInput").ap(),
        "wq": nc.dram_tensor("wq", [cfg.D, cfg.F], BF16, kind="ExternalInput").ap(),
        "wk": nc.dram_tensor("wk", [cfg.D, cfg.F], BF16, kind="ExternalInput").ap(),
        "wv": nc.dram_tensor("wv", [cfg.D, cfg.F], BF16, kind="ExternalInput").ap(),
        "wo": nc.dram_tensor("wo", [cfg.F, cfg.OUT], BF16, kind="ExternalInput").ap(),
        "wkey": nc.dram_tensor("wkey", [cfg.T], F32, kind="ExternalInput").ap(),
        "y": nc.dram_tensor("y", [cfg.T, cfg.OUT], F32, kind="ExternalOutput").ap(),
    }
    with tile.TileContext(nc) as tc:
        with ExitStack() as ctx:
            if n_iters == 1:
                emit(ctx, tc, io, cfg, blevel, do_c)
            else:
                with tc.For_i(0, n_iters, 1):
                    emit(ctx, tc, io, cfg, blevel, do_c)
    nc.compile()
    return nc


def shard_inputs(Q_tokens, K_tokens, V_tokens, weight_K, Wq, Wk, Wv, Wo, bo):
    """Build the 8 per-core input maps (full -> sharded, host side)."""
    import ml_dtypes
    bf = ml_dtypes.bfloat16
    scale = 1.0 / np.sqrt(np.float32(HD))
    Wq_s = (np.asarray(Wq, np.float32) * scale)
    in_maps = []
    qt = np.ascontiguousarray(np.asarray(Q_tokens, np.float32).transpose(0, 2, 1)).astype(bf)
    kt = np.ascontiguousarray(np.asarray(K_tokens, np.float32).transpose(0, 2, 1)).astype(bf)
    vt = np.ascontiguousarray(np.asarray(V_tokens, np.float32).transpose(0, 2, 1)).astype(bf)
    for c in range(N_CORES):
        b, g = divmod(c, 2)
        fs = slice(g * 512, (g + 1) * 512)
        in_maps.append({
            "xq": qt[b],
            "xk": kt[b],
            "xv": vt[b],
            "wq": np.ascontiguousarray(Wq_s[:, fs]).astype(bf),
            "wk": np.ascontiguousarray(np.asarray(Wk, np.float32)[:, fs]).astype(bf),
            "wv": np.ascontiguousarray(np.asarray(Wv, np.float32)[:, fs]).astype(bf),
            "wo": np.ascontiguousarray(np.asarray(Wo, np.float32)[fs, :]).astype(bf),
            "wkey": np.ascontiguousarray(np.asarray(weight_K, np.float32)[b]),
        })
    return in_maps


_PROGRAM_CACHE: dict = {}


def _get_program(n_iters: int = 1, blevel: int = 4, do_c: bool = True):
    key = (n_iters, blevel, do_c)
    if key not in _PROGRAM_CACHE:
        _PROGRAM_CACHE[key] = build_program(FULL, n_iters, blevel, do_c)
    return _PROGRAM_CACHE[key]


def run_sharded(in_maps, n_iters: int = 1):
    nc = _get_program(n_iters)
    t0 = time.time()
    res = run_bass_kernel_spmd(nc, in_maps, core_ids=list(range(N_CORES)))
    wall = time.time() - t0
    return res, wall


def kernel(Q_tokens, K_tokens, V_tokens, weight_K, Wq, Wk, Wv, Wo, bo):
    in_maps = shard_inputs(Q_tokens, K_tokens, V_tokens, weight_K,
                           Wq, Wk, Wv, Wo, bo)
    res, _ = run_sharded(in_maps)
    B = np.asarray(Q_tokens).shape[0]
    T, OUT = FULL.T, FULL.OUT
    out = np.empty((B, T, OUT), np.float32)
    bo = np.asarray(bo, np.float32)
    for b in range(B):
        out[b] = res.results[2 * b]["y"] + res.results[2 * b + 1]["y"] + bo
    return out
